# revision 28
# baseline (speedup 1.0000x reference)
"""Batched zero-phase Butterworth lowpass (filtfilt) on Trainium2.

The per-row map x -> y is linear; the two-sided impulse response g decays
as r^|d| (r ~ 0.82), negligible beyond |d| = 64. Each row is computed as a
banded Toeplitz matmul in bf16 (the 2e-2 rel-err budget dwarfs bf16 noise):
output tiles of 128 samples are produced by 2 accumulating PE matmuls
against 128-sample transposed input windows, with exact edge matrices for
the first/last tile (built numerically on host from b, a, zi). IO runs in
bf16, halving HBM traffic vs f32. Rows are sharded 512 per NeuronCore
across 8 cores.
"""

import sys

for _p in ("/opt/trn_rl_repo",):
    if _p not in sys.path:
        sys.path.insert(0, _p)

import ml_dtypes
import numpy as np

import concourse.bass as bass
import concourse.tile as tile
from concourse import bacc
from concourse import mybir
from concourse.bass_utils import run_bass_kernel_spmd

N = 8192
ROWS = 4096
NCORES = 8
RPC = ROWS // NCORES          # 512 rows per core
GROUPS = RPC // 128           # 4 groups of 128 rows
NT = N // 128                 # 64 output tiles of 128 per row
NW = NT + 1                   # 65 transposed input windows per row
W = 192                       # edge window with exact operator columns
PADLEN = 18

_mats_cache = {}
_nc_cache = {}


def _lfilter_batch(b, a, X, Zi):
    z = Zi.copy()
    Y = np.empty_like(X)
    b1, bm, bl = b[0], b[1:-1], b[-1]
    am, al = a[1:-1], a[-1]
    for t in range(X.shape[1]):
        xt = X[:, t]
        y = b1 * xt + z[:, 0]
        Y[:, t] = y
        z[:, :-1] = z[:, 1:] + np.outer(xt, bm) - np.outer(y, am)
        z[:, -1] = bl * xt - al * y
    return Y


def _filtfilt_batch(b, a, zi, X):
    left = 2 * X[:, :1] - X[:, PADLEN:0:-1]
    right = 2 * X[:, -1:] - X[:, -2:-(PADLEN + 2):-1]
    ext = np.concatenate([left, X, right], axis=1)
    y = _lfilter_batch(b, a, ext, np.outer(ext[:, 0], zi))
    y = _lfilter_batch(b, a, y[:, ::-1], np.outer(y[:, -1], zi))[:, ::-1]
    return y[:, PADLEN:-PADLEN]


def _build_mats(b, a, zi):
    """6 rhs matrices [128, 128] bf16: interior band pair (G0, G1) and the
    exact first/last-tile pairs (S00, S01) / (SL0, SL1)."""
    key = (b.tobytes(), a.tobytes(), zi.tobytes())
    if key in _mats_cache:
        return _mats_cache[key]
    b64, a64, zi64 = (np.asarray(v, np.float64) for v in (b, a, zi))

    j0 = N // 2
    basis = np.zeros((2 * W + 1, N))
    for i in range(W):
        basis[i, i] = 1.0
        basis[W + i, N - W + i] = 1.0
    basis[2 * W, j0] = 1.0
    cols = _filtfilt_batch(b64, a64, zi64, basis)
    g = cols[2 * W]                # M[:, j0]; band value g[d] = g[j0 + d]
    Mleft = cols[:W]               # Mleft[j, t] = M[t, j], j < W
    Mright = cols[W:2 * W]         # Mright[i, t] = M[t, N - W + i]

    def gband(d):
        dd = np.clip(j0 + d, 0, N - 1)
        out = g[dd]
        out[np.abs(d) > 150] = 0.0
        return out

    c = np.arange(128)[:, None]
    f = np.arange(128)[None, :]
    # tile k, window k   (x[128k-64, 128k+64)): weight[c, f] = g(c - 64 - f)
    # tile k, window k+1 (x[128k+64, 128k+192)): weight[c, f] = g(c + 64 - f)
    G0 = gband(c - 64 - f)
    G1 = gband(c + 64 - f)
    # first tile: exact M[f, j]; window 0 has j = c - 64 (c >= 64), window 1
    # has j = c + 64
    S00 = np.zeros((128, 128))
    S00[64:, :] = Mleft[0:64, 0:128]
    S01 = Mleft[64:192, 0:128]
    # last tile (outputs 8064 + f): window 63 has j = 8000 + c, window 64 has
    # j = 8128 + c (c < 64); N - W = 8000
    SL0 = Mright[0:128, N - 128:N]
    SL1 = np.zeros((128, 128))
    SL1[:64, :] = Mright[128:192, N - 128:N]

    wts = np.stack([G0, G1, S00, S01, SL0, SL1]).astype(ml_dtypes.bfloat16)
    # partition-major [128, 6*128] so the device DMA is 128 big descriptors
    wts = np.ascontiguousarray(wts.transpose(1, 0, 2).reshape(128, 6 * 128))
    _mats_cache[key] = wts
    return wts


def _build_nc():
    if "nc" in _nc_cache:
        return _nc_cache["nc"]
    f32 = mybir.dt.float32
    bf16 = mybir.dt.bfloat16
    nc = bacc.Bacc()
    x_in = nc.declare_dram_parameter("x", [RPC, N], bf16, isOutput=False)
    wts_in = nc.declare_dram_parameter("wts", [128, 6 * 128], bf16,
                                       isOutput=False)
    idn_in = nc.declare_dram_parameter("idn", [128, 128], bf16,
                                       isOutput=False)
    y_out = nc.declare_dram_parameter("y", [RPC, N], bf16, isOutput=True)

    half = N // 2
    with tile.TileContext(nc) as tc:
        with (
            tc.tile_pool(name="const", bufs=1) as constp,
            tc.tile_pool(name="xp", bufs=2) as xp,
            tc.tile_pool(name="xtp", bufs=2) as xtp,
            tc.tile_pool(name="outp", bufs=2) as outp,
            tc.tile_pool(name="pst", bufs=3, space="PSUM") as pst,
            tc.tile_pool(name="psc", bufs=4, space="PSUM") as psc,
        ):
            ident = constp.tile([128, 128], bf16, tag="ident")
            wt_all = constp.tile([128, 6 * 128], bf16, tag="wt_all")
            WG0, WG1, WS00, WS01, WSL0, WSL1 = [
                wt_all[:, i * 128:(i + 1) * 128] for i in range(6)
            ]

            for gidx in range(GROUPS):
                r0 = gidx * 128
                xpad = xp.tile([128, 64 + N + 64], bf16, tag="xpad")
                nc.gpsimd.memset(xpad[:, 0:64], 0.0)
                nc.gpsimd.memset(xpad[:, 64 + N:], 0.0)
                # quarter-chunk loads so transposes start early in each group
                nchunk = 4
                qc = N // nchunk
                for ci in range(nchunk):
                    nc.sync.dma_start(
                        xpad[:, 64 + ci * qc:64 + (ci + 1) * qc],
                        x_in[r0:r0 + 128, ci * qc:(ci + 1) * qc],
                    )
                    if gidx == 0 and ci == 0:
                        # constants gate the first transposes/matmuls: ride
                        # right behind chunk 0
                        nc.sync.dma_start(ident[:, :], idn_in[:, :])
                        nc.sync.dma_start(wt_all[:, :], wts_in[:, :])
                xt = xtp.tile([128, NW * 128], bf16, tag="xt")
                outbuf = outp.tile([128, N], bf16, tag="outbuf")

                state = {"nb": 0}

                def ensure_windows(upto_j, state=state, xpad=xpad, xt=xt):
                    # transpose 128-sample windows 4 at a time through PSUM
                    while state["nb"] * 4 <= upto_j:
                        q = state["nb"]
                        jb = 4 * q
                        nw = min(4, NW - jb)
                        pt = pst.tile([128, 512], bf16, tag="pt")
                        for s in range(nw):
                            j = jb + s
                            nc.tensor.transpose(
                                pt[:, 128 * s:128 * (s + 1)],
                                xpad[:, 128 * j:128 * (j + 1)],
                                ident[:, :],
                            )
                        nc.vector.tensor_copy(
                            xt[:, 128 * jb:128 * (jb + nw)],
                            pt[:, :128 * nw],
                        )
                        state["nb"] += 1

                # 8 output tiles per iteration: one long matmul run per
                # transpose run minimizes PE transpose<->matmul mode switches
                for mg in range(8):
                    ensure_windows(8 * mg + 8)
                    pcs = [psc.tile([128, 512], f32, tag="pc",
                                    name=f"pc{h}")
                           for h in range(2)]
                    for i in range(8):
                        k = 8 * mg + i
                        if k == 0:
                            w0, w1 = WS00, WS01
                        elif k == NT - 1:
                            w0, w1 = WSL0, WSL1
                        else:
                            w0, w1 = WG0, WG1
                        pc = pcs[i // 4]
                        o = 128 * (i % 4)
                        nc.tensor.matmul(
                            pc[:, o:o + 128],
                            xt[:, 128 * k:128 * (k + 1)], w0,
                            start=True, stop=False,
                        )
                        nc.tensor.matmul(
                            pc[:, o:o + 128],
                            xt[:, 128 * (k + 1):128 * (k + 2)], w1,
                            start=False, stop=True,
                        )
                    c0 = 1024 * mg
                    nc.scalar.copy(outbuf[:, c0:c0 + 512], pcs[0][:, :])
                    nc.vector.tensor_copy(outbuf[:, c0 + 512:c0 + 1024],
                                          pcs[1][:, :])
                    if gidx == GROUPS - 1:
                        # last group: store each 1024 as soon as it drains
                        # so the tail after the final drain is short
                        nc.scalar.dma_start(y_out[r0:r0 + 128, c0:c0 + 1024],
                                            outbuf[:, c0:c0 + 1024])
                    elif mg % 2 == 1:
                        c1 = 2048 * (mg // 2)
                        nc.scalar.dma_start(y_out[r0:r0 + 128, c1:c1 + 2048],
                                            outbuf[:, c1:c1 + 2048])
    nc.compile()
    _nc_cache["nc"] = nc
    return nc


def _run(inputs, trace=False, trace_kwargs=None):
    x = np.asarray(inputs["x"])
    b = np.asarray(inputs["b"], np.float32)
    a = np.asarray(inputs["a"], np.float32)
    zi = np.asarray(inputs["zi"], np.float32)
    wts = _build_mats(b, a, zi)
    x_bf = np.ascontiguousarray(x.astype(ml_dtypes.bfloat16))
    idn = np.eye(128, dtype=ml_dtypes.bfloat16)
    nc = _build_nc()
    in_maps = [
        {"x": x_bf[i * RPC:(i + 1) * RPC], "wts": wts, "idn": idn}
        for i in range(NCORES)
    ]
    res = run_bass_kernel_spmd(
        nc, in_maps, list(range(NCORES)), trace=trace,
        **(trace_kwargs or {}),
    )
    y = np.concatenate(
        [res.results[i]["y"].astype(np.float32) for i in range(NCORES)],
        axis=0,
    )
    return y, res


def kernel(**inputs) -> np.ndarray:
    y, _ = _run(inputs, trace=False)
    return y


# revision 29
# speedup vs baseline: 1.1558x; 1.1558x over previous
"""Batched zero-phase Butterworth lowpass (filtfilt) on Trainium2.

The per-row map x -> y is linear; the two-sided impulse response g decays
as r^|d| (r ~ 0.82), negligible beyond |d| = 64. Each row is computed as a
banded Toeplitz matmul in bf16 (the 2e-2 rel-err budget dwarfs bf16 noise):
output tiles of 128 samples are produced by 2 accumulating PE matmuls
against 128-sample transposed input windows, with exact edge matrices for
the first/last tile (built numerically on host from b, a, zi). IO runs in
bf16, halving HBM traffic vs f32. Rows are sharded 512 per NeuronCore
across 8 cores.
"""

import sys

for _p in ("/opt/trn_rl_repo",):
    if _p not in sys.path:
        sys.path.insert(0, _p)

import ml_dtypes
import numpy as np

import concourse.bass as bass
import concourse.tile as tile
from concourse import bacc
from concourse import mybir
from concourse.bass_utils import run_bass_kernel_spmd

N = 8192
ROWS = 4096
NCORES = 8
RPC = ROWS // NCORES          # 512 rows per core
GROUPS = RPC // 128           # 4 groups of 128 rows
NT = N // 128                 # 64 output tiles of 128 per row
NW = NT + 1                   # 65 transposed input windows per row
W = 192                       # edge window with exact operator columns
PADLEN = 18

_mats_cache = {}
_nc_cache = {}


def _lfilter_batch(b, a, X, Zi):
    z = Zi.copy()
    Y = np.empty_like(X)
    b1, bm, bl = b[0], b[1:-1], b[-1]
    am, al = a[1:-1], a[-1]
    for t in range(X.shape[1]):
        xt = X[:, t]
        y = b1 * xt + z[:, 0]
        Y[:, t] = y
        z[:, :-1] = z[:, 1:] + np.outer(xt, bm) - np.outer(y, am)
        z[:, -1] = bl * xt - al * y
    return Y


def _filtfilt_batch(b, a, zi, X):
    left = 2 * X[:, :1] - X[:, PADLEN:0:-1]
    right = 2 * X[:, -1:] - X[:, -2:-(PADLEN + 2):-1]
    ext = np.concatenate([left, X, right], axis=1)
    y = _lfilter_batch(b, a, ext, np.outer(ext[:, 0], zi))
    y = _lfilter_batch(b, a, y[:, ::-1], np.outer(y[:, -1], zi))[:, ::-1]
    return y[:, PADLEN:-PADLEN]


def _build_mats(b, a, zi):
    """6 rhs matrices [128, 128] bf16: interior band pair (G0, G1) and the
    exact first/last-tile pairs (S00, S01) / (SL0, SL1)."""
    key = (b.tobytes(), a.tobytes(), zi.tobytes())
    if key in _mats_cache:
        return _mats_cache[key]
    b64, a64, zi64 = (np.asarray(v, np.float64) for v in (b, a, zi))

    j0 = N // 2
    basis = np.zeros((2 * W + 1, N))
    for i in range(W):
        basis[i, i] = 1.0
        basis[W + i, N - W + i] = 1.0
    basis[2 * W, j0] = 1.0
    cols = _filtfilt_batch(b64, a64, zi64, basis)
    g = cols[2 * W]                # M[:, j0]; band value g[d] = g[j0 + d]
    Mleft = cols[:W]               # Mleft[j, t] = M[t, j], j < W
    Mright = cols[W:2 * W]         # Mright[i, t] = M[t, N - W + i]

    def gband(d):
        dd = np.clip(j0 + d, 0, N - 1)
        out = g[dd]
        out[np.abs(d) > 150] = 0.0
        return out

    c = np.arange(128)[:, None]
    f = np.arange(128)[None, :]
    # tile k, window k   (x[128k-64, 128k+64)): weight[c, f] = g(c - 64 - f)
    # tile k, window k+1 (x[128k+64, 128k+192)): weight[c, f] = g(c + 64 - f)
    G0 = gband(c - 64 - f)
    G1 = gband(c + 64 - f)
    # first tile: exact M[f, j]; window 0 has j = c - 64 (c >= 64), window 1
    # has j = c + 64
    S00 = np.zeros((128, 128))
    S00[64:, :] = Mleft[0:64, 0:128]
    S01 = Mleft[64:192, 0:128]
    # last tile (outputs 8064 + f): window 63 has j = 8000 + c, window 64 has
    # j = 8128 + c (c < 64); N - W = 8000
    SL0 = Mright[0:128, N - 128:N]
    SL1 = np.zeros((128, 128))
    SL1[:64, :] = Mright[128:192, N - 128:N]

    wts = np.stack([G0, G1, S00, S01, SL0, SL1]).astype(ml_dtypes.bfloat16)
    # partition-major [128, 6*128] so the device DMA is 128 big descriptors
    wts = np.ascontiguousarray(wts.transpose(1, 0, 2).reshape(128, 6 * 128))
    _mats_cache[key] = wts
    return wts


def _build_nc():
    if "nc" in _nc_cache:
        return _nc_cache["nc"]
    f32 = mybir.dt.float32
    bf16 = mybir.dt.bfloat16
    nc = bacc.Bacc()
    x_in = nc.declare_dram_parameter("x", [RPC, N], bf16, isOutput=False)
    wts_in = nc.declare_dram_parameter("wts", [128, 6 * 128], bf16,
                                       isOutput=False)
    idn_in = nc.declare_dram_parameter("idn", [128, 128], bf16,
                                       isOutput=False)
    y_out = nc.declare_dram_parameter("y", [RPC, N], bf16, isOutput=True)

    half = N // 2
    with tile.TileContext(nc) as tc:
        with (
            tc.tile_pool(name="const", bufs=1) as constp,
            tc.tile_pool(name="xp", bufs=2) as xp,
            tc.tile_pool(name="xtp", bufs=2) as xtp,
            tc.tile_pool(name="outp", bufs=2) as outp,
            tc.tile_pool(name="warm", bufs=1, space="PSUM") as warmp,
            tc.tile_pool(name="pst", bufs=3, space="PSUM") as pst,
            tc.tile_pool(name="psc", bufs=4, space="PSUM") as psc,
        ):
            # PE p-state warmup: ~24 throwaway transposes on a zeroed tile
            # during the initial DMA window, so real work runs at 2.4 GHz
            scratch = constp.tile([128, 128], bf16, tag="scratch")
            nc.gpsimd.memset(scratch[:, :], 0.0)
            wps = warmp.tile([128, 512], bf16, tag="wps")
            for wi in range(24):
                nc.tensor.transpose(
                    wps[:, 128 * (wi % 4):128 * (wi % 4 + 1)],
                    scratch[:, :], scratch[:, :],
                )
            ident = constp.tile([128, 128], bf16, tag="ident")
            wt_all = constp.tile([128, 6 * 128], bf16, tag="wt_all")
            WG0, WG1, WS00, WS01, WSL0, WSL1 = [
                wt_all[:, i * 128:(i + 1) * 128] for i in range(6)
            ]

            for gidx in range(GROUPS):
                r0 = gidx * 128
                xpad = xp.tile([128, 64 + N + 64], bf16, tag="xpad")
                nc.gpsimd.memset(xpad[:, 0:64], 0.0)
                nc.gpsimd.memset(xpad[:, 64 + N:], 0.0)
                # quarter-chunk loads so transposes start early in each group
                nchunk = 4
                qc = N // nchunk
                for ci in range(nchunk):
                    nc.sync.dma_start(
                        xpad[:, 64 + ci * qc:64 + (ci + 1) * qc],
                        x_in[r0:r0 + 128, ci * qc:(ci + 1) * qc],
                    )
                    if gidx == 0 and ci == 0:
                        # constants gate the first transposes/matmuls: ride
                        # right behind chunk 0
                        nc.sync.dma_start(ident[:, :], idn_in[:, :])
                        nc.sync.dma_start(wt_all[:, :], wts_in[:, :])
                xt = xtp.tile([128, NW * 128], bf16, tag="xt")
                outbuf = outp.tile([128, N], bf16, tag="outbuf")

                state = {"nb": 0}

                def ensure_windows(upto_j, state=state, xpad=xpad, xt=xt):
                    # transpose 128-sample windows 4 at a time through PSUM
                    while state["nb"] * 4 <= upto_j:
                        q = state["nb"]
                        jb = 4 * q
                        nw = min(4, NW - jb)
                        pt = pst.tile([128, 512], bf16, tag="pt")
                        for s in range(nw):
                            j = jb + s
                            nc.tensor.transpose(
                                pt[:, 128 * s:128 * (s + 1)],
                                xpad[:, 128 * j:128 * (j + 1)],
                                ident[:, :],
                            )
                        nc.vector.tensor_copy(
                            xt[:, 128 * jb:128 * (jb + nw)],
                            pt[:, :128 * nw],
                        )
                        state["nb"] += 1

                # 8 output tiles per iteration: one long matmul run per
                # transpose run minimizes PE transpose<->matmul mode switches
                for mg in range(8):
                    ensure_windows(8 * mg + 8)
                    pcs = [psc.tile([128, 512], f32, tag="pc",
                                    name=f"pc{h}")
                           for h in range(2)]
                    for i in range(8):
                        k = 8 * mg + i
                        if k == 0:
                            w0, w1 = WS00, WS01
                        elif k == NT - 1:
                            w0, w1 = WSL0, WSL1
                        else:
                            w0, w1 = WG0, WG1
                        pc = pcs[i // 4]
                        o = 128 * (i % 4)
                        nc.tensor.matmul(
                            pc[:, o:o + 128],
                            xt[:, 128 * k:128 * (k + 1)], w0,
                            start=True, stop=False,
                        )
                        nc.tensor.matmul(
                            pc[:, o:o + 128],
                            xt[:, 128 * (k + 1):128 * (k + 2)], w1,
                            start=False, stop=True,
                        )
                    c0 = 1024 * mg
                    nc.scalar.copy(outbuf[:, c0:c0 + 512], pcs[0][:, :])
                    nc.vector.tensor_copy(outbuf[:, c0 + 512:c0 + 1024],
                                          pcs[1][:, :])
                    if gidx == GROUPS - 1:
                        # last group: store each 1024 as soon as it drains
                        # so the tail after the final drain is short
                        nc.scalar.dma_start(y_out[r0:r0 + 128, c0:c0 + 1024],
                                            outbuf[:, c0:c0 + 1024])
                    elif mg % 2 == 1:
                        c1 = 2048 * (mg // 2)
                        nc.scalar.dma_start(y_out[r0:r0 + 128, c1:c1 + 2048],
                                            outbuf[:, c1:c1 + 2048])
    nc.compile()
    _nc_cache["nc"] = nc
    return nc


def _run(inputs, trace=False, trace_kwargs=None):
    x = np.asarray(inputs["x"])
    b = np.asarray(inputs["b"], np.float32)
    a = np.asarray(inputs["a"], np.float32)
    zi = np.asarray(inputs["zi"], np.float32)
    wts = _build_mats(b, a, zi)
    x_bf = np.ascontiguousarray(x.astype(ml_dtypes.bfloat16))
    idn = np.eye(128, dtype=ml_dtypes.bfloat16)
    nc = _build_nc()
    in_maps = [
        {"x": x_bf[i * RPC:(i + 1) * RPC], "wts": wts, "idn": idn}
        for i in range(NCORES)
    ]
    res = run_bass_kernel_spmd(
        nc, in_maps, list(range(NCORES)), trace=trace,
        **(trace_kwargs or {}),
    )
    y = np.concatenate(
        [res.results[i]["y"].astype(np.float32) for i in range(NCORES)],
        axis=0,
    )
    return y, res


def kernel(**inputs) -> np.ndarray:
    y, _ = _run(inputs, trace=False)
    return y
